# revision 37
# baseline (speedup 1.0000x reference)
"""Binary conv (BN -> sign -> binarized 3x3 conv -> bias -> relu) on 8 TRN2 cores.

Strategy
--------
Data-parallel over batch: each of the 8 NeuronCores gets 8 of the 64 images.

  phase P (prologue):  load w early (amortizes into the phase-A stream);
                       sign() -> bf16 and PE-transpose into fp8e4 DoubleRow
                       lhsT layout EMITTED AFTER phase A, so the ACT work
                       queues behind the stats squares (w-signs emitted first
                       delay stat completion and stall the x stream).
  phase A (stats):     stream x shard as 16 fat 1.6MB contiguous DMAs; DVE
                       reduce_sum accumulates per-channel sums, ScalarE
                       Square+accum_out per-channel sum-of-squares. Image 0's
                       tiles stay resident in SBUF for phase B. One [128,4]
                       fp32 AllReduce across the 8 cores (cc_in written via
                       the fast HWDGE sync queue so the trigger fires ~2us
                       after stats); then scale_c = gamma_c * rsqrt(var_c+eps),
                       shift_c = beta_c - mean_c*scale_c.
  phase B (conv):      per image: ACT computes sign(scale*x + shift) -> fp8e4
                       into a zero-padded flat [58*58] SBUF plane (image 0
                       from the resident tiles with row-split signs, so the
                       first conv blocks launch ~3us earlier; images 1+
                       re-streamed behind a sync-FIFO gate on cc_out — any
                       DMA during the collective window steals HBM-stack
                       bandwidth from the stack-mate still streaming stats
                       and inflates the rendezvous skew);
                       conv as 9 taps x fp8 DoubleRow matmuls (contracting
                       all 256 ci at once) into [128co x 464px] PSUM tiles
                       over contiguous 8-row windows (the 2 wrap columns are
                       computed and discarded); DVE fuses +bias and relu on
                       the PSUM evacuation; DMA out.

sign() outputs +-1 exactly representable in fp8e4, PE accumulates in fp32
(integer sums bounded by 2304), so the conv arithmetic is exact.

Measured: the conv phase is PE-bound at ~96% of the fp8-DoubleRow roofline
(1008 back-to-back 226ns matmuls, zero gaps); phase A is HBM-bound at the
~270GB/s per-core rate both HBM-stack-mates sustain together. The remaining
slack is the ncfw AllReduce (11us CC-core bootstrap + run-varying 5-25us
cross-core rendezvous skew).
"""

import os
import sys

import numpy as np

for _p in ("/opt/trn_rl_repo", "/root/.axon_site/_ro/trn_rl_repo"):
    if os.path.isdir(_p) and _p not in sys.path:
        sys.path.append(_p)

import concourse.bass as bass
import concourse.bacc as bacc
import concourse.tile as tile
from concourse import mybir
from concourse.bass_utils import run_bass_kernel_spmd
from concourse.masks import make_identity

AF = mybir.ActivationFunctionType
ALU = mybir.AluOpType
F32 = mybir.dt.float32
BF16 = mybir.dt.bfloat16
FP8 = mybir.dt.float8e4

N_CORES = 8
N_IMG = 8          # images per core
C = 256            # channels (in == out)
H = W = 56
HW = H * W         # 3136
PW = W + 2         # 58 padded
PLANE = PW * PW    # 3364
# plane data at offset 1 (1 guard elem before, guards/pad after); padded so the
# DoubleRow pair stride (N_IMG//2 * PLANE_G fp8 elements) is a multiple of 16
PLANE_G = PLANE + 4  # 3368
EPS = 1e-5
N_TOTAL = 64 * HW  # BN reduction count over full batch
ROWS_PER_BLK = 8
N_BLK = H // ROWS_PER_BLK        # 7
BLK_FREE = ROWS_PER_BLK * PW     # 464 (incl. 2 wrap columns/row)
OUT_FREE = ROWS_PER_BLK * W      # 448 valid outputs

_CACHE = {}


def _build_nc():
    nc = bacc.Bacc(None, target_bir_lowering=False, num_devices=N_CORES)

    x_d = nc.dram_tensor("x", [N_IMG, C, HW], F32, kind="ExternalInput")
    g_d = nc.dram_tensor("gamma", [C], F32, kind="ExternalInput")
    be_d = nc.dram_tensor("beta", [C], F32, kind="ExternalInput")
    w_d = nc.dram_tensor("w", [C, C * 9], F32, kind="ExternalInput")
    b_d = nc.dram_tensor("b", [C], F32, kind="ExternalInput")
    y_d = nc.dram_tensor("y", [N_IMG, C, HW], F32, kind="ExternalOutput")
    # single [128,4] stats AllReduce (ncfw serializes queued collectives and
    # each pays a skew-dominated rendezvous, so splitting per-chunk is a loss)
    cc_in = nc.dram_tensor("cc_in", [128, 4], F32)
    cc_out = nc.dram_tensor("cc_out", [128, 4], F32, addr_space="Shared")
    # dummy warm-up collective ran at t~0: ncfw serializes collectives, so it
    # completes under phase A and pre-warms the CC-core mesh path for the
    # real stats AllReduce (whose ~11us bootstrap is on the critical path)
    warm_in = nc.dram_tensor("warm_in", [128, 1], F32)
    warm_out = nc.dram_tensor("warm_out", [128, 1], F32, addr_space="Shared")

    with tile.TileContext(nc) as tc:
        with (
            tc.tile_pool(name="persist", bufs=1) as persist,
            tc.tile_pool(name="keep", bufs=1) as keep_pool,
            tc.tile_pool(name="xin", bufs=6) as xin_pool,
            tc.tile_pool(name="outp", bufs=4) as out_pool,
            tc.tile_pool(name="vec", bufs=1) as vec_pool,
        ):
            # image 0 stays resident in SBUF from the phase-A stream, so
            # after the AllReduce the first conv needs ZERO new DMA —
            # sign reads these tiles directly while images 1+ re-stream.
            N_KEEP = 1
            keep_tiles = [
                [
                    keep_pool.tile([128, HW], F32, name=f"xkeep_{n}_{c}")
                    for c in range(2)
                ]
                for n in range(N_KEEP)
            ]
            # padded+binarized activations, split by image parity so sign()
            # writes for image n+1 don't WAR-serialize against conv reads of
            # image n: [ci_part, ci_pair(j), img//2, guarded flat plane]
            xpadA = persist.tile([128, 2, N_IMG // 2, PLANE_G], FP8)
            xpadB = persist.tile([128, 2, N_IMG // 2, PLANE_G], FP8)
            xpads = [xpadA, xpadB]
            # conv weights, fp8 DoubleRow lhsT layout: [ci_part, tap, co_chunk, j, co]
            wt = persist.tile([128, 9, 2, 2, 128], FP8)

            # per-channel vectors, [128, 2] = (partition, ci_chunk); on the
            # gpsimd DGE so the sync-engine FIFO is left to the x/y streams
            gamma_sb = vec_pool.tile([128, 2], F32)
            beta_sb = vec_pool.tile([128, 2], F32)
            bias_sb = vec_pool.tile([128, 2], F32)
            nc.gpsimd.dma_start(gamma_sb, g_d.rearrange("(c p) -> p c", p=128))
            nc.gpsimd.dma_start(beta_sb, be_d.rearrange("(c p) -> p c", p=128))
            nc.gpsimd.dma_start(bias_sb, b_d.rearrange("(c p) -> p c", p=128))

            nc.gpsimd.collective_compute(
                "AllReduce",
                ALU.add,
                replica_groups=[list(range(N_CORES))],
                ins=[warm_in[:]],
                outs=[warm_out[:]],
            )

            # phase P part 1: weight DMA + identity, at kernel start (the
            # 2.4MB wf load amortizes into the phase-A stream on every core
            # symmetrically; the ACT sign / PE transpose half of phase P is
            # emitted AFTER phase A so it queues behind the stats squares)
            ident = vec_pool.tile([128, 128], BF16)
            make_identity(nc, ident)
            ws = xin_pool.tile([128, 2, C * 9], BF16, tag="xt")
            wfs = []
            for o in range(2):
                wf = xin_pool.tile([128, C * 9], F32, tag="xt", name=f"wf_{o}")
                nc.gpsimd.dma_start(wf, w_d[o * 128 : (o + 1) * 128, :])
                wfs.append(wf)

            # zero borders (rows 0/57, cols 0/57 of each plane) + guard elements
            # (plane data starts at flat offset 1; offset 0 / PLANE+1 are
            # guards) — emitted first: no deps, runs on DVE in the first ~15us
            xrows = []
            for xp in xpads:
                xrow = xp[:, :, :, 1 : 1 + PLANE].rearrange(
                    "p j n (r c) -> p j n r c", c=PW
                )
                xrows.append(xrow)
                nc.vector.memset(xrow[:, :, :, 0, :], 0.0)
                nc.vector.memset(xrow[:, :, :, PW - 1, :], 0.0)
                nc.vector.memset(xrow[:, :, :, :, 0], 0.0)
                nc.vector.memset(xrow[:, :, :, :, PW - 1], 0.0)
                nc.vector.memset(xp[:, :, :, 0:1], 0.0)
                nc.vector.memset(xp[:, :, :, PLANE + 1 : PLANE_G], 0.0)

            # ---------------- phase A: BN stats (x stream starts immediately
            # on the sync DGE). One fat 1.6MB DMA per (image, ci-chunk) —
            # 12.5KB contiguous per partition line keeps HBM near line rate.
            # Reductions fire per chunk as each DMA lands.
            sums = vec_pool.tile([128, 2, N_IMG], F32)
            sumsq = vec_pool.tile([128, 2, N_IMG], F32)
            cc_sb = vec_pool.tile([128, 2, 2], F32)   # per chunk: (sum, sumsq)
            with tc.tile_pool(name="trash", bufs=1) as trash_pool:
                for n in range(N_IMG):
                    for c in range(2):
                        if n < N_KEEP:
                            xt = keep_tiles[n][c]
                        else:
                            xt = xin_pool.tile([128, HW], F32)
                        nc.sync.dma_start(
                            xt, x_d[n, c * 128 : (c + 1) * 128, :]
                        )
                        nc.vector.reduce_sum(
                            sums[:, c, n : n + 1], xt,
                            axis=mybir.AxisListType.X,
                        )
                        tr = trash_pool.tile([128, HW], F32)
                        nc.scalar.activation(
                            tr, xt, AF.Square,
                            accum_out=sumsq[:, c, n : n + 1],
                        )
                for c in range(2):
                    nc.vector.reduce_sum(
                        cc_sb[:, c, 0:1], sums[:, c, :],
                        axis=mybir.AxisListType.X,
                    )
                    nc.vector.reduce_sum(
                        cc_sb[:, c, 1:2], sumsq[:, c, :],
                        axis=mybir.AxisListType.X,
                    )

            # cc_in write rides the fast HWDGE sync queue: it dispatches the
            # moment the stats consolidation lands (no SWDGE ~1us + FIFO
            # skew), and everything queued behind it on sync (the phase-B x
            # prefetches) streams during the collective's rendezvous window.
            nc.sync.dma_start(cc_in[:], cc_sb.rearrange("p a b -> p (a b)"))
            nc.gpsimd.collective_compute(
                "AllReduce",
                ALU.add,
                replica_groups=[list(range(N_CORES))],
                ins=[cc_in[:]],
                outs=[cc_out[:]],
            )

            # ---------------- phase P part 2: w-sign + transpose (emitted
            # after phase A so the ACT w-signs queue BEHIND the stats squares
            # on the scalar FIFO — emitted early they delay stat completion
            # and stall the stream; PE is idle here and wt is needed ~30us
            # after this point, so the latency is fully hidden)
            with tc.tile_pool(name="wps", bufs=2, space="PSUM") as wps:
                for o in range(2):
                    nc.scalar.activation(ws[:, o, :], wfs[o], AF.Sign)
                ws_r = ws.rearrange("p o (ci tap) -> p o ci tap", tap=9)
                for t in range(9):
                    for c in range(2):
                        for o in range(2):
                            pw = wps.tile([128, 128], BF16)
                            nc.tensor.transpose(
                                pw, ws_r[:, o, c * 128 : (c + 1) * 128, t], ident
                            )
                            nc.vector.tensor_copy(wt[:, t, o, c, :], pw)

            gl = vec_pool.tile([128, 2, 2], F32)
            nc.gpsimd.dma_start(gl.rearrange("p a b -> p (a b)"), cc_out[:])

            # per-chunk finalize: scale_c = gamma_c / sqrt(var_c + eps),
            # shift_c = beta_c - mean_c * scale_c. Abs_reciprocal_sqrt's loose
            # precision only scales scl's magnitude (scl stays > 0), which
            # sign() cannot observe — outputs remain exact.
            eps_sb = vec_pool.tile([128, 1], F32)
            nc.vector.memset(eps_sb, EPS)
            mean = vec_pool.tile([128, 2], F32)
            m2 = vec_pool.tile([128, 2], F32)
            var = vec_pool.tile([128, 2], F32)
            rstd = vec_pool.tile([128, 2], F32)
            scl = vec_pool.tile([128, 2], F32)
            sh = vec_pool.tile([128, 2], F32)
            for c in range(2):
                cs = slice(c, c + 1)
                nc.vector.tensor_scalar_mul(
                    mean[:, cs], gl[:, c, 0:1], 1.0 / N_TOTAL
                )
                nc.vector.tensor_tensor(
                    m2[:, cs], mean[:, cs], mean[:, cs], op=ALU.mult
                )
                nc.vector.scalar_tensor_tensor(
                    out=var[:, cs],
                    in0=gl[:, c, 1:2],
                    scalar=1.0 / N_TOTAL,
                    in1=m2[:, cs],
                    op0=ALU.mult,
                    op1=ALU.subtract,
                )
                nc.scalar.activation(
                    rstd[:, cs], var[:, cs], AF.Abs_reciprocal_sqrt,
                    bias=eps_sb[:],
                )
                nc.vector.tensor_mul(scl[:, cs], gamma_sb[:, cs], rstd[:, cs])
                nc.vector.tensor_mul(sh[:, cs], mean[:, cs], scl[:, cs])
                nc.vector.tensor_sub(sh[:, cs], beta_sb[:, cs], sh[:, cs])

            # sync-FIFO gate: holds the phase-B x re-reads until the AllReduce
            # completes. Any DMA issued during the collective window steals
            # HBM-stack bandwidth from the stack-mate core still streaming
            # its stats AND from the collective's own mesh transfers
            # (measured: ungated prefetch grew the peer wait 5.5us -> 25us).
            gate_sb = vec_pool.tile([128, 4], F32)
            nc.sync.dma_start(gate_sb, cc_out[:])

            # ---------------- phase B: sign + conv ----------------
            with tc.tile_pool(name="cps", bufs=8, space="PSUM") as cps:
                for n in range(N_IMG):
                    xp = xpads[n % 2]
                    slot = n // 2
                    # image 0 signs in row-halves interleaved across the two
                    # ci chunks: the first conv blocks need rows <=9 of BOTH
                    # chunks, so they launch after ~2.9us instead of ~5.8us
                    # (subtile deps gate each block on exactly the row range
                    # it reads). Later images pipeline ahead of the conv, so
                    # the single-call form is fine there.
                    row_splits = [(0, 28), (28, 56)] if n == 0 else [(0, 56)]
                    xts = []
                    for c in range(2):
                        if n < N_KEEP:
                            xt = keep_tiles[n][c]
                        else:
                            xt = xin_pool.tile([128, HW], F32)
                            nc.sync.dma_start(
                                xt, x_d[n, c * 128 : (c + 1) * 128, :]
                            )
                        xts.append(xt.rearrange("p (h w) -> p h w", w=W))
                    for r0, r1 in row_splits:
                        for c in range(2):
                            nc.scalar.activation(
                                xrows[n % 2][
                                    :, c, slot, 1 + r0 : 1 + r1, 1 : W + 1
                                ],
                                xts[c][:, r0:r1, :],
                                AF.Sign,
                                bias=sh[:, c : c + 1],
                                scale=scl[:, c : c + 1],
                            )
                    for o in range(2):
                        for bi in range(N_BLK):
                            ps = cps.tile([128, BLK_FREE], F32)
                            r0 = bi * ROWS_PER_BLK
                            for t in range(9):
                                ky, kx = divmod(t, 3)
                                base = 1 + (r0 + ky) * PW + (kx - 1)
                                nc.tensor.matmul(
                                    ps,
                                    wt[:, t, o],
                                    xp[:, :, slot, base : base + BLK_FREE],
                                    start=(t == 0),
                                    stop=(t == 8),
                                    perf_mode=mybir.MatmulPerfMode.DoubleRow,
                                )
                            ob = out_pool.tile([128, OUT_FREE], F32)
                            # relu(psum + bias): (x + b) then max(.., 0) on DVE,
                            # dropping the 2 wrap columns of each row
                            nc.vector.tensor_scalar(
                                out=ob,
                                in0=ps.rearrange("p (r c) -> p r c", c=PW)[
                                    :, :, 1 : W + 1
                                ],
                                scalar1=bias_sb[:, o : o + 1],
                                scalar2=0.0,
                                op0=ALU.add,
                                op1=ALU.max,
                            )
                            nc.sync.dma_start(
                                y_d[
                                    n, o * 128 : (o + 1) * 128,
                                    bi * OUT_FREE : (bi + 1) * OUT_FREE,
                                ],
                                ob,
                            )

    nc.finalize()
    return nc


def get_nc():
    if "nc" not in _CACHE:
        _CACHE["nc"] = _build_nc()
    return _CACHE["nc"]


def run(x, gamma, beta, w, b, trace=False, trace_cores=None):
    x = np.ascontiguousarray(np.asarray(x, dtype=np.float32))
    gamma = np.ascontiguousarray(np.asarray(gamma, dtype=np.float32))
    beta = np.ascontiguousarray(np.asarray(beta, dtype=np.float32))
    w = np.ascontiguousarray(np.asarray(w, dtype=np.float32)).reshape(C, C * 9)
    b = np.ascontiguousarray(np.asarray(b, dtype=np.float32))

    nc = get_nc()
    in_maps = []
    for i in range(N_CORES):
        in_maps.append(
            {
                "x": np.ascontiguousarray(
                    x[i * N_IMG : (i + 1) * N_IMG].reshape(N_IMG, C, HW)
                ),
                "gamma": gamma,
                "beta": beta,
                "w": w,
                "b": b,
            }
        )
    res = run_bass_kernel_spmd(
        nc, in_maps, list(range(N_CORES)), trace=trace, trace_cores=trace_cores
    )
    y = np.concatenate(
        [r["y"].reshape(N_IMG, C, H, W) for r in res.results], axis=0
    )
    return y.astype(np.float32), res


def kernel(x, gamma, beta, w, b):
    y, _ = run(x, gamma, beta, w, b, trace=False)
    return y



# revision 38
# speedup vs baseline: 1.0754x; 1.0754x over previous
"""Binary conv (BN -> sign -> binarized 3x3 conv -> bias -> relu) on 8 TRN2 cores.

Strategy
--------
Data-parallel over batch: each of the 8 NeuronCores gets 8 of the 64 images.

  phase P (prologue):  load w early (amortizes into the phase-A stream);
                       sign() -> bf16 and PE-transpose into fp8e4 DoubleRow
                       lhsT layout EMITTED AFTER phase A, so the ACT work
                       queues behind the stats squares (w-signs emitted first
                       delay stat completion and stall the x stream).
  phase A (stats):     stream x shard as 16 fat 1.6MB contiguous DMAs; DVE
                       reduce_sum accumulates per-channel sums, ScalarE
                       Square+accum_out per-channel sum-of-squares. Image 0's
                       tiles stay resident in SBUF for phase B. One [128,4]
                       fp32 AllReduce across the 8 cores (cc_in written via
                       the fast HWDGE sync queue so the trigger fires ~2us
                       after stats); then scale_c = gamma_c * rsqrt(var_c+eps),
                       shift_c = beta_c - mean_c*scale_c.
  phase B (conv):      per image: ACT computes sign(scale*x + shift) -> fp8e4
                       into a zero-padded flat [58*58] SBUF plane (image 0
                       from the resident tiles with row-split signs, so the
                       first conv blocks launch ~3us earlier; images 1+
                       re-streamed behind a sync-FIFO gate on cc_out — any
                       DMA during the collective window steals HBM-stack
                       bandwidth from the stack-mate still streaming stats
                       and inflates the rendezvous skew);
                       conv as 9 taps x fp8 DoubleRow matmuls (contracting
                       all 256 ci at once) into [128co x 464px] PSUM tiles
                       over contiguous 8-row windows (the 2 wrap columns are
                       computed and discarded); DVE fuses +bias and relu on
                       the PSUM evacuation; DMA out.

sign() outputs +-1 exactly representable in fp8e4, PE accumulates in fp32
(integer sums bounded by 2304), so the conv arithmetic is exact.

Measured: the conv phase is PE-bound at ~96% of the fp8-DoubleRow roofline
(1008 back-to-back 226ns matmuls, zero gaps); phase A is HBM-bound at the
~270GB/s per-core rate both HBM-stack-mates sustain together. The remaining
slack is the ncfw AllReduce (11us CC-core bootstrap + run-varying 5-25us
cross-core rendezvous skew).
"""

import os
import sys

import numpy as np

for _p in ("/opt/trn_rl_repo", "/root/.axon_site/_ro/trn_rl_repo"):
    if os.path.isdir(_p) and _p not in sys.path:
        sys.path.append(_p)

import concourse.bass as bass
import concourse.bacc as bacc
import concourse.tile as tile
from concourse import mybir
from concourse.bass_utils import run_bass_kernel_spmd
from concourse.masks import make_identity

AF = mybir.ActivationFunctionType
ALU = mybir.AluOpType
F32 = mybir.dt.float32
BF16 = mybir.dt.bfloat16
FP8 = mybir.dt.float8e4

N_CORES = 8
N_IMG = 8          # images per core
C = 256            # channels (in == out)
H = W = 56
HW = H * W         # 3136
PW = W + 2         # 58 padded
PLANE = PW * PW    # 3364
# plane data at offset 1 (1 guard elem before, guards/pad after); padded so the
# DoubleRow pair stride (N_IMG//2 * PLANE_G fp8 elements) is a multiple of 16
PLANE_G = PLANE + 4  # 3368
EPS = 1e-5
N_TOTAL = 64 * HW  # BN reduction count over full batch
ROWS_PER_BLK = 8
N_BLK = H // ROWS_PER_BLK        # 7
BLK_FREE = ROWS_PER_BLK * PW     # 464 (incl. 2 wrap columns/row)
OUT_FREE = ROWS_PER_BLK * W      # 448 valid outputs

_CACHE = {}


def _build_nc():
    nc = bacc.Bacc(None, target_bir_lowering=False, num_devices=N_CORES)

    x_d = nc.dram_tensor("x", [N_IMG, C, HW], F32, kind="ExternalInput")
    g_d = nc.dram_tensor("gamma", [C], F32, kind="ExternalInput")
    be_d = nc.dram_tensor("beta", [C], F32, kind="ExternalInput")
    w_d = nc.dram_tensor("w", [C, C * 9], F32, kind="ExternalInput")
    b_d = nc.dram_tensor("b", [C], F32, kind="ExternalInput")
    y_d = nc.dram_tensor("y", [N_IMG, C, HW], F32, kind="ExternalOutput")
    # single [128,4] stats AllReduce (ncfw serializes queued collectives and
    # each pays a skew-dominated rendezvous, so splitting per-chunk is a loss)
    cc_in = nc.dram_tensor("cc_in", [128, 4], F32)
    cc_out = nc.dram_tensor("cc_out", [128, 4], F32, addr_space="Shared")

    with tile.TileContext(nc) as tc:
        with (
            tc.tile_pool(name="persist", bufs=1) as persist,
            tc.tile_pool(name="keep", bufs=1) as keep_pool,
            tc.tile_pool(name="xin", bufs=6) as xin_pool,
            tc.tile_pool(name="outp", bufs=4) as out_pool,
            tc.tile_pool(name="vec", bufs=1) as vec_pool,
        ):
            # image 0 stays resident in SBUF from the phase-A stream, so
            # after the AllReduce the first conv needs ZERO new DMA —
            # sign reads these tiles directly while images 1+ re-stream.
            N_KEEP = 1
            keep_tiles = [
                [
                    keep_pool.tile([128, HW], F32, name=f"xkeep_{n}_{c}")
                    for c in range(2)
                ]
                for n in range(N_KEEP)
            ]
            # padded+binarized activations, split by image parity so sign()
            # writes for image n+1 don't WAR-serialize against conv reads of
            # image n: [ci_part, ci_pair(j), img//2, guarded flat plane]
            xpadA = persist.tile([128, 2, N_IMG // 2, PLANE_G], FP8)
            xpadB = persist.tile([128, 2, N_IMG // 2, PLANE_G], FP8)
            xpads = [xpadA, xpadB]
            # conv weights, fp8 DoubleRow lhsT layout: [ci_part, tap, co_chunk, j, co]
            wt = persist.tile([128, 9, 2, 2, 128], FP8)

            # per-channel vectors, [128, 2] = (partition, ci_chunk); on the
            # gpsimd DGE so the sync-engine FIFO is left to the x/y streams
            gamma_sb = vec_pool.tile([128, 2], F32)
            beta_sb = vec_pool.tile([128, 2], F32)
            bias_sb = vec_pool.tile([128, 2], F32)
            nc.gpsimd.dma_start(gamma_sb, g_d.rearrange("(c p) -> p c", p=128))
            nc.gpsimd.dma_start(beta_sb, be_d.rearrange("(c p) -> p c", p=128))
            nc.gpsimd.dma_start(bias_sb, b_d.rearrange("(c p) -> p c", p=128))

            # phase P part 1: weight DMA + identity, at kernel start (the
            # 2.4MB wf load amortizes into the phase-A stream on every core
            # symmetrically; the ACT sign / PE transpose half of phase P is
            # emitted AFTER phase A so it queues behind the stats squares)
            ident = vec_pool.tile([128, 128], BF16)
            make_identity(nc, ident)
            ws = xin_pool.tile([128, 2, C * 9], BF16, tag="xt")
            wfs = []
            for o in range(2):
                wf = xin_pool.tile([128, C * 9], F32, tag="xt", name=f"wf_{o}")
                nc.gpsimd.dma_start(wf, w_d[o * 128 : (o + 1) * 128, :])
                wfs.append(wf)

            # zero borders (rows 0/57, cols 0/57 of each plane) + guard elements
            # (plane data starts at flat offset 1; offset 0 / PLANE+1 are
            # guards) — emitted first: no deps, runs on DVE in the first ~15us
            xrows = []
            for xp in xpads:
                xrow = xp[:, :, :, 1 : 1 + PLANE].rearrange(
                    "p j n (r c) -> p j n r c", c=PW
                )
                xrows.append(xrow)
                nc.vector.memset(xrow[:, :, :, 0, :], 0.0)
                nc.vector.memset(xrow[:, :, :, PW - 1, :], 0.0)
                nc.vector.memset(xrow[:, :, :, :, 0], 0.0)
                nc.vector.memset(xrow[:, :, :, :, PW - 1], 0.0)
                nc.vector.memset(xp[:, :, :, 0:1], 0.0)
                nc.vector.memset(xp[:, :, :, PLANE + 1 : PLANE_G], 0.0)

            # ---------------- phase A: BN stats (x stream starts immediately
            # on the sync DGE). One fat 1.6MB DMA per (image, ci-chunk) —
            # 12.5KB contiguous per partition line keeps HBM near line rate.
            # Reductions fire per chunk as each DMA lands.
            sums = vec_pool.tile([128, 2, N_IMG], F32)
            sumsq = vec_pool.tile([128, 2, N_IMG], F32)
            cc_sb = vec_pool.tile([128, 2, 2], F32)   # per chunk: (sum, sumsq)
            with tc.tile_pool(name="trash", bufs=1) as trash_pool:
                for n in range(N_IMG):
                    for c in range(2):
                        if n < N_KEEP:
                            xt = keep_tiles[n][c]
                        else:
                            xt = xin_pool.tile([128, HW], F32)
                        nc.sync.dma_start(
                            xt, x_d[n, c * 128 : (c + 1) * 128, :]
                        )
                        nc.vector.reduce_sum(
                            sums[:, c, n : n + 1], xt,
                            axis=mybir.AxisListType.X,
                        )
                        tr = trash_pool.tile([128, HW], F32)
                        nc.scalar.activation(
                            tr, xt, AF.Square,
                            accum_out=sumsq[:, c, n : n + 1],
                        )
                for c in range(2):
                    nc.vector.reduce_sum(
                        cc_sb[:, c, 0:1], sums[:, c, :],
                        axis=mybir.AxisListType.X,
                    )
                    nc.vector.reduce_sum(
                        cc_sb[:, c, 1:2], sumsq[:, c, :],
                        axis=mybir.AxisListType.X,
                    )

            # cc_in write rides the fast HWDGE sync queue: it dispatches the
            # moment the stats consolidation lands (no SWDGE ~1us + FIFO
            # skew), and everything queued behind it on sync (the phase-B x
            # prefetches) streams during the collective's rendezvous window.
            nc.sync.dma_start(cc_in[:], cc_sb.rearrange("p a b -> p (a b)"))
            nc.gpsimd.collective_compute(
                "AllReduce",
                ALU.add,
                replica_groups=[list(range(N_CORES))],
                ins=[cc_in[:]],
                outs=[cc_out[:]],
            )

            # ---------------- phase P part 2: w-sign + transpose (emitted
            # after phase A so the ACT w-signs queue BEHIND the stats squares
            # on the scalar FIFO — emitted early they delay stat completion
            # and stall the stream; PE is idle here and wt is needed ~30us
            # after this point, so the latency is fully hidden)
            with tc.tile_pool(name="wps", bufs=2, space="PSUM") as wps:
                for o in range(2):
                    nc.scalar.activation(ws[:, o, :], wfs[o], AF.Sign)
                ws_r = ws.rearrange("p o (ci tap) -> p o ci tap", tap=9)
                for t in range(9):
                    for c in range(2):
                        for o in range(2):
                            pw = wps.tile([128, 128], BF16)
                            nc.tensor.transpose(
                                pw, ws_r[:, o, c * 128 : (c + 1) * 128, t], ident
                            )
                            nc.vector.tensor_copy(wt[:, t, o, c, :], pw)

            gl = vec_pool.tile([128, 2, 2], F32)
            nc.gpsimd.dma_start(gl.rearrange("p a b -> p (a b)"), cc_out[:])

            # per-chunk finalize: scale_c = gamma_c / sqrt(var_c + eps),
            # shift_c = beta_c - mean_c * scale_c. Abs_reciprocal_sqrt's loose
            # precision only scales scl's magnitude (scl stays > 0), which
            # sign() cannot observe — outputs remain exact.
            eps_sb = vec_pool.tile([128, 1], F32)
            nc.vector.memset(eps_sb, EPS)
            mean = vec_pool.tile([128, 2], F32)
            m2 = vec_pool.tile([128, 2], F32)
            var = vec_pool.tile([128, 2], F32)
            rstd = vec_pool.tile([128, 2], F32)
            scl = vec_pool.tile([128, 2], F32)
            sh = vec_pool.tile([128, 2], F32)
            for c in range(2):
                cs = slice(c, c + 1)
                nc.vector.tensor_scalar_mul(
                    mean[:, cs], gl[:, c, 0:1], 1.0 / N_TOTAL
                )
                nc.vector.tensor_tensor(
                    m2[:, cs], mean[:, cs], mean[:, cs], op=ALU.mult
                )
                nc.vector.scalar_tensor_tensor(
                    out=var[:, cs],
                    in0=gl[:, c, 1:2],
                    scalar=1.0 / N_TOTAL,
                    in1=m2[:, cs],
                    op0=ALU.mult,
                    op1=ALU.subtract,
                )
                nc.scalar.activation(
                    rstd[:, cs], var[:, cs], AF.Abs_reciprocal_sqrt,
                    bias=eps_sb[:],
                )
                nc.vector.tensor_mul(scl[:, cs], gamma_sb[:, cs], rstd[:, cs])
                nc.vector.tensor_mul(sh[:, cs], mean[:, cs], scl[:, cs])
                nc.vector.tensor_sub(sh[:, cs], beta_sb[:, cs], sh[:, cs])

            # sync-FIFO gate: holds the phase-B x re-reads until the AllReduce
            # completes. Any DMA issued during the collective window steals
            # HBM-stack bandwidth from the stack-mate core still streaming
            # its stats AND from the collective's own mesh transfers
            # (measured: ungated prefetch grew the peer wait 5.5us -> 25us).
            gate_sb = vec_pool.tile([128, 4], F32)
            nc.sync.dma_start(gate_sb, cc_out[:])

            # ---------------- phase B: sign + conv ----------------
            with tc.tile_pool(name="cps", bufs=8, space="PSUM") as cps:
                for n in range(N_IMG):
                    xp = xpads[n % 2]
                    slot = n // 2
                    # image 0 signs in row-halves interleaved across the two
                    # ci chunks: the first conv blocks need rows <=9 of BOTH
                    # chunks, so they launch after ~2.9us instead of ~5.8us
                    # (subtile deps gate each block on exactly the row range
                    # it reads). Later images pipeline ahead of the conv, so
                    # the single-call form is fine there.
                    row_splits = [(0, 28), (28, 56)] if n == 0 else [(0, 56)]
                    xts = []
                    for c in range(2):
                        if n < N_KEEP:
                            xt = keep_tiles[n][c]
                        else:
                            xt = xin_pool.tile([128, HW], F32)
                            nc.sync.dma_start(
                                xt, x_d[n, c * 128 : (c + 1) * 128, :]
                            )
                        xts.append(xt.rearrange("p (h w) -> p h w", w=W))
                    for r0, r1 in row_splits:
                        for c in range(2):
                            nc.scalar.activation(
                                xrows[n % 2][
                                    :, c, slot, 1 + r0 : 1 + r1, 1 : W + 1
                                ],
                                xts[c][:, r0:r1, :],
                                AF.Sign,
                                bias=sh[:, c : c + 1],
                                scale=scl[:, c : c + 1],
                            )
                    for o in range(2):
                        for bi in range(N_BLK):
                            ps = cps.tile([128, BLK_FREE], F32)
                            r0 = bi * ROWS_PER_BLK
                            for t in range(9):
                                ky, kx = divmod(t, 3)
                                base = 1 + (r0 + ky) * PW + (kx - 1)
                                nc.tensor.matmul(
                                    ps,
                                    wt[:, t, o],
                                    xp[:, :, slot, base : base + BLK_FREE],
                                    start=(t == 0),
                                    stop=(t == 8),
                                    perf_mode=mybir.MatmulPerfMode.DoubleRow,
                                )
                            ob = out_pool.tile([128, OUT_FREE], F32)
                            # relu(psum + bias): (x + b) then max(.., 0) on DVE,
                            # dropping the 2 wrap columns of each row
                            nc.vector.tensor_scalar(
                                out=ob,
                                in0=ps.rearrange("p (r c) -> p r c", c=PW)[
                                    :, :, 1 : W + 1
                                ],
                                scalar1=bias_sb[:, o : o + 1],
                                scalar2=0.0,
                                op0=ALU.add,
                                op1=ALU.max,
                            )
                            nc.sync.dma_start(
                                y_d[
                                    n, o * 128 : (o + 1) * 128,
                                    bi * OUT_FREE : (bi + 1) * OUT_FREE,
                                ],
                                ob,
                            )

    nc.finalize()
    return nc


def get_nc():
    if "nc" not in _CACHE:
        _CACHE["nc"] = _build_nc()
    return _CACHE["nc"]


def run(x, gamma, beta, w, b, trace=False, trace_cores=None):
    x = np.ascontiguousarray(np.asarray(x, dtype=np.float32))
    gamma = np.ascontiguousarray(np.asarray(gamma, dtype=np.float32))
    beta = np.ascontiguousarray(np.asarray(beta, dtype=np.float32))
    w = np.ascontiguousarray(np.asarray(w, dtype=np.float32)).reshape(C, C * 9)
    b = np.ascontiguousarray(np.asarray(b, dtype=np.float32))

    nc = get_nc()
    in_maps = []
    for i in range(N_CORES):
        in_maps.append(
            {
                "x": np.ascontiguousarray(
                    x[i * N_IMG : (i + 1) * N_IMG].reshape(N_IMG, C, HW)
                ),
                "gamma": gamma,
                "beta": beta,
                "w": w,
                "b": b,
            }
        )
    res = run_bass_kernel_spmd(
        nc, in_maps, list(range(N_CORES)), trace=trace, trace_cores=trace_cores
    )
    y = np.concatenate(
        [r["y"].reshape(N_IMG, C, H, W) for r in res.results], axis=0
    )
    return y.astype(np.float32), res


def kernel(x, gamma, beta, w, b):
    y, _ = run(x, gamma, beta, w, b, trace=False)
    return y



# revision 40
# speedup vs baseline: 1.0832x; 1.0073x over previous
"""Binary conv (BN -> sign -> binarized 3x3 conv -> bias -> relu) on 8 TRN2 cores.

Strategy
--------
Data-parallel over batch: each of the 8 NeuronCores gets 8 of the 64 images.

  phase P (prologue):  load w early (amortizes into the phase-A stream);
                       sign() -> bf16 and PE-transpose into fp8e4 DoubleRow
                       lhsT layout EMITTED AFTER phase A, so the ACT work
                       queues behind the stats squares (w-signs emitted first
                       delay stat completion and stall the x stream).
  phase A (stats):     stream x shard as 16 fat 1.6MB contiguous DMAs; DVE
                       reduce_sum accumulates per-channel sums, ScalarE
                       Square+accum_out per-channel sum-of-squares. Image 0's
                       tiles stay resident in SBUF for phase B. One [128,4]
                       fp32 AllReduce across the 8 cores (cc_in written via
                       the fast HWDGE sync queue so the trigger fires ~2us
                       after stats); then scale_c = gamma_c * rsqrt(var_c+eps),
                       shift_c = beta_c - mean_c*scale_c.
  phase B (conv):      per image: ACT computes sign(scale*x + shift) -> fp8e4
                       into a zero-padded flat [58*58] SBUF plane (image 0
                       from the resident tiles with row-split signs, so the
                       first conv blocks launch ~3us earlier; images 1+
                       re-streamed behind a sync-FIFO gate on cc_out — any
                       DMA during the collective window steals HBM-stack
                       bandwidth from the stack-mate still streaming stats
                       and inflates the rendezvous skew);
                       conv as 9 taps x fp8 DoubleRow matmuls (contracting
                       all 256 ci at once) into [128co x 464px] PSUM tiles
                       over contiguous 8-row windows (the 2 wrap columns are
                       computed and discarded); DVE fuses +bias and relu on
                       the PSUM evacuation; DMA out.

sign() outputs +-1 exactly representable in fp8e4, PE accumulates in fp32
(integer sums bounded by 2304), so the conv arithmetic is exact.

Measured: the conv phase is PE-bound at ~96% of the fp8-DoubleRow roofline
(1008 back-to-back 226ns matmuls, zero gaps); phase A is HBM-bound at the
~270GB/s per-core rate both HBM-stack-mates sustain together. The remaining
slack is the ncfw AllReduce (11us CC-core bootstrap + run-varying 5-25us
cross-core rendezvous skew).
"""

import os
import sys

import numpy as np

for _p in ("/opt/trn_rl_repo", "/root/.axon_site/_ro/trn_rl_repo"):
    if os.path.isdir(_p) and _p not in sys.path:
        sys.path.append(_p)

import concourse.bass as bass
import concourse.bacc as bacc
import concourse.tile as tile
from concourse import mybir
from concourse.bass_utils import run_bass_kernel_spmd
from concourse.masks import make_identity

AF = mybir.ActivationFunctionType
ALU = mybir.AluOpType
F32 = mybir.dt.float32
BF16 = mybir.dt.bfloat16
FP8 = mybir.dt.float8e4

N_CORES = 8
N_IMG = 8          # images per core
C = 256            # channels (in == out)
H = W = 56
HW = H * W         # 3136
PW = W + 2         # 58 padded
PLANE = PW * PW    # 3364
# plane data at offset 1 (1 guard elem before, guards/pad after); padded so the
# DoubleRow pair stride (N_IMG//2 * PLANE_G fp8 elements) is a multiple of 16
PLANE_G = PLANE + 4  # 3368
EPS = 1e-5
N_TOTAL = 64 * HW  # BN reduction count over full batch
ROWS_PER_BLK = 8
N_BLK = H // ROWS_PER_BLK        # 7
BLK_FREE = ROWS_PER_BLK * PW     # 464 (incl. 2 wrap columns/row)
OUT_FREE = ROWS_PER_BLK * W      # 448 valid outputs

_CACHE = {}


def _build_nc():
    nc = bacc.Bacc(None, target_bir_lowering=False, num_devices=N_CORES)

    x_d = nc.dram_tensor("x", [N_IMG, C, HW], F32, kind="ExternalInput")
    g_d = nc.dram_tensor("gamma", [C], F32, kind="ExternalInput")
    be_d = nc.dram_tensor("beta", [C], F32, kind="ExternalInput")
    w_d = nc.dram_tensor("w", [C, C * 9], F32, kind="ExternalInput")
    b_d = nc.dram_tensor("b", [C], F32, kind="ExternalInput")
    y_d = nc.dram_tensor("y", [N_IMG, C, HW], F32, kind="ExternalOutput")
    # single [128,4] stats AllReduce (ncfw serializes queued collectives and
    # each pays a skew-dominated rendezvous, so splitting per-chunk is a loss)
    cc_in = nc.dram_tensor("cc_in", [128, 4], F32)
    cc_out = nc.dram_tensor("cc_out", [128, 4], F32, addr_space="Shared")
    # dummy warm-up collective at t~0: ncfw serializes collectives, so it
    # completes under phase A and pre-warms the CC-core mesh path — the real
    # stats AllReduce's TPB_TRIGGER->ALGO_MESH_BEGIN drops ~11us -> ~1.2us
    # (measured). Output unused; input garbage is fine.
    warm_in = nc.dram_tensor("warm_in", [128, 1], F32)
    warm_out = nc.dram_tensor("warm_out", [128, 1], F32, addr_space="Shared")

    with tile.TileContext(nc) as tc:
        with (
            tc.tile_pool(name="persist", bufs=1) as persist,
            tc.tile_pool(name="keep", bufs=1) as keep_pool,
            tc.tile_pool(name="xin", bufs=6) as xin_pool,
            tc.tile_pool(name="outp", bufs=4) as out_pool,
            tc.tile_pool(name="vec", bufs=1) as vec_pool,
        ):
            # image 0 stays resident in SBUF from the phase-A stream, so
            # after the AllReduce the first conv needs ZERO new DMA —
            # sign reads these tiles directly while images 1+ re-stream.
            N_KEEP = 1
            keep_tiles = [
                [
                    keep_pool.tile([128, HW], F32, name=f"xkeep_{n}_{c}")
                    for c in range(2)
                ]
                for n in range(N_KEEP)
            ]
            # padded+binarized activations, split by image parity so sign()
            # writes for image n+1 don't WAR-serialize against conv reads of
            # image n: [ci_part, ci_pair(j), img//2, guarded flat plane]
            xpadA = persist.tile([128, 2, N_IMG // 2, PLANE_G], FP8)
            xpadB = persist.tile([128, 2, N_IMG // 2, PLANE_G], FP8)
            xpads = [xpadA, xpadB]
            # conv weights, fp8 DoubleRow lhsT layout: [ci_part, tap, co_chunk, j, co]
            wt = persist.tile([128, 9, 2, 2, 128], FP8)

            # per-channel vectors, [128, 2] = (partition, ci_chunk); on the
            # gpsimd DGE so the sync-engine FIFO is left to the x/y streams
            gamma_sb = vec_pool.tile([128, 2], F32)
            beta_sb = vec_pool.tile([128, 2], F32)
            bias_sb = vec_pool.tile([128, 2], F32)
            nc.gpsimd.dma_start(gamma_sb, g_d.rearrange("(c p) -> p c", p=128))
            nc.gpsimd.dma_start(beta_sb, be_d.rearrange("(c p) -> p c", p=128))
            nc.gpsimd.dma_start(bias_sb, b_d.rearrange("(c p) -> p c", p=128))

            nc.gpsimd.collective_compute(
                "AllReduce",
                ALU.add,
                replica_groups=[list(range(N_CORES))],
                ins=[warm_in[:]],
                outs=[warm_out[:]],
            )

            # phase P part 1: weight DMA + identity, at kernel start (the
            # 2.4MB wf load amortizes into the phase-A stream on every core
            # symmetrically; the ACT sign / PE transpose half of phase P is
            # emitted AFTER phase A so it queues behind the stats squares)
            ident = vec_pool.tile([128, 128], BF16)
            make_identity(nc, ident)
            ws = xin_pool.tile([128, 2, C * 9], BF16, tag="xt")
            wfs = []
            for o in range(2):
                wf = xin_pool.tile([128, C * 9], F32, tag="xt", name=f"wf_{o}")
                nc.gpsimd.dma_start(wf, w_d[o * 128 : (o + 1) * 128, :])
                wfs.append(wf)

            # zero borders (rows 0/57, cols 0/57 of each plane) + guard elements
            # (plane data starts at flat offset 1; offset 0 / PLANE+1 are
            # guards) — emitted first: no deps, runs on DVE in the first ~15us
            xrows = []
            for xp in xpads:
                xrow = xp[:, :, :, 1 : 1 + PLANE].rearrange(
                    "p j n (r c) -> p j n r c", c=PW
                )
                xrows.append(xrow)
                nc.vector.memset(xrow[:, :, :, 0, :], 0.0)
                nc.vector.memset(xrow[:, :, :, PW - 1, :], 0.0)
                nc.vector.memset(xrow[:, :, :, :, 0], 0.0)
                nc.vector.memset(xrow[:, :, :, :, PW - 1], 0.0)
                nc.vector.memset(xp[:, :, :, 0:1], 0.0)
                nc.vector.memset(xp[:, :, :, PLANE + 1 : PLANE_G], 0.0)

            # ---------------- phase A: BN stats (x stream starts immediately
            # on the sync DGE). One fat 1.6MB DMA per (image, ci-chunk) —
            # 12.5KB contiguous per partition line keeps HBM near line rate.
            # Reductions fire per chunk as each DMA lands.
            sums = vec_pool.tile([128, 2, N_IMG], F32)
            sumsq = vec_pool.tile([128, 2, N_IMG], F32)
            cc_sb = vec_pool.tile([128, 2, 2], F32)   # per chunk: (sum, sumsq)
            with tc.tile_pool(name="trash", bufs=1) as trash_pool:
                for n in range(N_IMG):
                    for c in range(2):
                        if n < N_KEEP:
                            xt = keep_tiles[n][c]
                        else:
                            xt = xin_pool.tile([128, HW], F32)
                        nc.sync.dma_start(
                            xt, x_d[n, c * 128 : (c + 1) * 128, :]
                        )
                        nc.vector.reduce_sum(
                            sums[:, c, n : n + 1], xt,
                            axis=mybir.AxisListType.X,
                        )
                        tr = trash_pool.tile([128, HW], F32)
                        nc.scalar.activation(
                            tr, xt, AF.Square,
                            accum_out=sumsq[:, c, n : n + 1],
                        )
                for c in range(2):
                    nc.vector.reduce_sum(
                        cc_sb[:, c, 0:1], sums[:, c, :],
                        axis=mybir.AxisListType.X,
                    )
                    nc.vector.reduce_sum(
                        cc_sb[:, c, 1:2], sumsq[:, c, :],
                        axis=mybir.AxisListType.X,
                    )

            # cc_in write rides the fast HWDGE sync queue: it dispatches the
            # moment the stats consolidation lands (no SWDGE ~1us + FIFO
            # skew), and everything queued behind it on sync (the phase-B x
            # prefetches) streams during the collective's rendezvous window.
            nc.sync.dma_start(cc_in[:], cc_sb.rearrange("p a b -> p (a b)"))
            nc.gpsimd.collective_compute(
                "AllReduce",
                ALU.add,
                replica_groups=[list(range(N_CORES))],
                ins=[cc_in[:]],
                outs=[cc_out[:]],
            )

            # ---------------- phase P part 2: w-sign + transpose (emitted
            # after phase A so the ACT w-signs queue BEHIND the stats squares
            # on the scalar FIFO — emitted early they delay stat completion
            # and stall the stream; PE is idle here and wt is needed ~30us
            # after this point, so the latency is fully hidden)
            with tc.tile_pool(name="wps", bufs=2, space="PSUM") as wps:
                for o in range(2):
                    nc.scalar.activation(ws[:, o, :], wfs[o], AF.Sign)
                ws_r = ws.rearrange("p o (ci tap) -> p o ci tap", tap=9)
                for t in range(9):
                    for c in range(2):
                        for o in range(2):
                            pw = wps.tile([128, 128], BF16)
                            nc.tensor.transpose(
                                pw, ws_r[:, o, c * 128 : (c + 1) * 128, t], ident
                            )
                            nc.vector.tensor_copy(wt[:, t, o, c, :], pw)

            gl = vec_pool.tile([128, 2, 2], F32)
            nc.gpsimd.dma_start(gl.rearrange("p a b -> p (a b)"), cc_out[:])

            # per-chunk finalize: scale_c = gamma_c / sqrt(var_c + eps),
            # shift_c = beta_c - mean_c * scale_c. Abs_reciprocal_sqrt's loose
            # precision only scales scl's magnitude (scl stays > 0), which
            # sign() cannot observe — outputs remain exact.
            eps_sb = vec_pool.tile([128, 1], F32)
            nc.vector.memset(eps_sb, EPS)
            mean = vec_pool.tile([128, 2], F32)
            m2 = vec_pool.tile([128, 2], F32)
            var = vec_pool.tile([128, 2], F32)
            rstd = vec_pool.tile([128, 2], F32)
            scl = vec_pool.tile([128, 2], F32)
            sh = vec_pool.tile([128, 2], F32)
            for c in range(2):
                cs = slice(c, c + 1)
                nc.vector.tensor_scalar_mul(
                    mean[:, cs], gl[:, c, 0:1], 1.0 / N_TOTAL
                )
                nc.vector.tensor_tensor(
                    m2[:, cs], mean[:, cs], mean[:, cs], op=ALU.mult
                )
                nc.vector.scalar_tensor_tensor(
                    out=var[:, cs],
                    in0=gl[:, c, 1:2],
                    scalar=1.0 / N_TOTAL,
                    in1=m2[:, cs],
                    op0=ALU.mult,
                    op1=ALU.subtract,
                )
                nc.scalar.activation(
                    rstd[:, cs], var[:, cs], AF.Abs_reciprocal_sqrt,
                    bias=eps_sb[:],
                )
                nc.vector.tensor_mul(scl[:, cs], gamma_sb[:, cs], rstd[:, cs])
                nc.vector.tensor_mul(sh[:, cs], mean[:, cs], scl[:, cs])
                nc.vector.tensor_sub(sh[:, cs], beta_sb[:, cs], sh[:, cs])

            # sync-FIFO gate: holds the phase-B x re-reads until the AllReduce
            # completes. Any DMA issued during the collective window steals
            # HBM-stack bandwidth from the stack-mate core still streaming
            # its stats AND from the collective's own mesh transfers
            # (measured: ungated prefetch grew the peer wait 5.5us -> 25us).
            gate_sb = vec_pool.tile([128, 4], F32)
            nc.sync.dma_start(gate_sb, cc_out[:])

            # ---------------- phase B: sign + conv ----------------
            with tc.tile_pool(name="cps", bufs=8, space="PSUM") as cps:
                for n in range(N_IMG):
                    xp = xpads[n % 2]
                    slot = n // 2
                    # image 0 signs in row-halves interleaved across the two
                    # ci chunks: the first conv blocks need rows <=9 of BOTH
                    # chunks, so they launch after ~2.9us instead of ~5.8us
                    # (subtile deps gate each block on exactly the row range
                    # it reads). Later images pipeline ahead of the conv, so
                    # the single-call form is fine there.
                    row_splits = [(0, 28), (28, 56)] if n == 0 else [(0, 56)]
                    xts = []
                    for c in range(2):
                        if n < N_KEEP:
                            xt = keep_tiles[n][c]
                        else:
                            xt = xin_pool.tile([128, HW], F32)
                            nc.sync.dma_start(
                                xt, x_d[n, c * 128 : (c + 1) * 128, :]
                            )
                        xts.append(xt.rearrange("p (h w) -> p h w", w=W))
                    for r0, r1 in row_splits:
                        for c in range(2):
                            nc.scalar.activation(
                                xrows[n % 2][
                                    :, c, slot, 1 + r0 : 1 + r1, 1 : W + 1
                                ],
                                xts[c][:, r0:r1, :],
                                AF.Sign,
                                bias=sh[:, c : c + 1],
                                scale=scl[:, c : c + 1],
                            )
                    for o in range(2):
                        for bi in range(N_BLK):
                            ps = cps.tile([128, BLK_FREE], F32)
                            r0 = bi * ROWS_PER_BLK
                            for t in range(9):
                                ky, kx = divmod(t, 3)
                                base = 1 + (r0 + ky) * PW + (kx - 1)
                                nc.tensor.matmul(
                                    ps,
                                    wt[:, t, o],
                                    xp[:, :, slot, base : base + BLK_FREE],
                                    start=(t == 0),
                                    stop=(t == 8),
                                    perf_mode=mybir.MatmulPerfMode.DoubleRow,
                                )
                            ob = out_pool.tile([128, OUT_FREE], F32)
                            # relu(psum + bias): (x + b) then max(.., 0) on DVE,
                            # dropping the 2 wrap columns of each row
                            nc.vector.tensor_scalar(
                                out=ob,
                                in0=ps.rearrange("p (r c) -> p r c", c=PW)[
                                    :, :, 1 : W + 1
                                ],
                                scalar1=bias_sb[:, o : o + 1],
                                scalar2=0.0,
                                op0=ALU.add,
                                op1=ALU.max,
                            )
                            nc.sync.dma_start(
                                y_d[
                                    n, o * 128 : (o + 1) * 128,
                                    bi * OUT_FREE : (bi + 1) * OUT_FREE,
                                ],
                                ob,
                            )

    nc.finalize()
    return nc


def get_nc():
    if "nc" not in _CACHE:
        _CACHE["nc"] = _build_nc()
    return _CACHE["nc"]


def run(x, gamma, beta, w, b, trace=False, trace_cores=None):
    x = np.ascontiguousarray(np.asarray(x, dtype=np.float32))
    gamma = np.ascontiguousarray(np.asarray(gamma, dtype=np.float32))
    beta = np.ascontiguousarray(np.asarray(beta, dtype=np.float32))
    w = np.ascontiguousarray(np.asarray(w, dtype=np.float32)).reshape(C, C * 9)
    b = np.ascontiguousarray(np.asarray(b, dtype=np.float32))

    nc = get_nc()
    in_maps = []
    for i in range(N_CORES):
        in_maps.append(
            {
                "x": np.ascontiguousarray(
                    x[i * N_IMG : (i + 1) * N_IMG].reshape(N_IMG, C, HW)
                ),
                "gamma": gamma,
                "beta": beta,
                "w": w,
                "b": b,
            }
        )
    res = run_bass_kernel_spmd(
        nc, in_maps, list(range(N_CORES)), trace=trace, trace_cores=trace_cores
    )
    y = np.concatenate(
        [r["y"].reshape(N_IMG, C, H, W) for r in res.results], axis=0
    )
    return y.astype(np.float32), res


def kernel(x, gamma, beta, w, b):
    y, _ = run(x, gamma, beta, w, b, trace=False)
    return y

